# revision 36
# baseline (speedup 1.0000x reference)
"""Multi-head attention (B=4, S=2048, D=1024, H=16, causal + RoPE) on 8 trn2 cores.

Sharding: head-parallel. Core c owns heads {2c, 2c+1}:
  - Q/K/V projections for its 2 heads over all B*S positions,
  - RoPE + causal softmax attention,
  - row-parallel partial out-projection; host sums the 8 bf16 partials.

v2 design notes (instruction-count-driven; each matmul ~230ns fixed, each
dma_start ~625ns of serialized issue):
  - block pipeline: per (batch, 512-query-block): project q/k/v -> RoPE ->
    attention -> out-proj; block k's projection is emitted before block k-1's
    attention so PE never waits on RoPE.
  - RoPE rotate-half partners are placed on adjacent partitions by permuting
    the q/k weight columns host-side, so the half-swap is a single DVE
    stream_shuffle (mask [1,0,3,2,...]) instead of DMAs.
  - V is transposed into key-major vones layout with dma_start_transpose.
  - j-chunks processed in pairs: 2 score matmuls -> one [128,1024] exp -> 2 AV
    matmuls; diagonal chunks use column subranges; causal mask multiplies only
    the triangle strips.
  - softmax denominator via an extra vones column that carries the key-padding
    mask (1.0/0.0); the reciprocal row is broadcast across partitions with a
    K=1 matmul into PSUM instead of a broadcast DMA.
  - out partials written as bf16 (host accumulates in f32).
"""

import numpy as np

# ---- fixed problem config ----
B, S, D = 4, 2048, 1024
H, HD = 16, 64
NCORES = 8
HPC = H // NCORES          # heads per core = 2
ROPE_BASE = 10000.0

QBLK = 512                 # query block (free dim of scores/AV matmuls)
JCH = 128                  # key chunk (partition dim of scores^T)


# --------------------------------------------------------------------------
# host-side helpers
# --------------------------------------------------------------------------

def _perm64():
    """Rotate-half pairing permutation: head-dim i -> 2i, i+32 -> 2i+1."""
    p = np.empty(64, dtype=np.int64)
    p[np.arange(32) * 2] = np.arange(32)          # even slots <- dims 0..31
    p[np.arange(32) * 2 + 1] = np.arange(32, 64)  # odd slots  <- dims 32..63
    return p                                       # p[slot] = orig dim


def _rope_tables_T(s, hd, hpc):
    """cos/sin tables [hpc*hd, s] in permuted row order, sin sign-folded.

    After permutation, partition 2i holds dim i and partition 2i+1 holds dim
    i+32 (per head block of 64). rot-half swap = swap adjacent partitions;
    sign: even slots get -sin, odd slots +sin. cos/sin rows use freq of
    dim mod 32 (emb = concat(freqs, freqs)).
    """
    inv_freq = 1.0 / (ROPE_BASE ** (np.arange(0, hd, 2, dtype=np.float32) / np.float32(hd)))
    t = np.arange(s, dtype=np.float32)
    freqs = np.outer(t, inv_freq).astype(np.float32)          # [s, hd/2]
    emb = np.concatenate([freqs, freqs], axis=-1)             # [s, hd]
    cos = np.cos(emb).T.astype(np.float32)                    # [hd, s]
    sin = np.sin(emb).T.astype(np.float32)
    perm = _perm64()
    cosp = cos[perm]                                          # [hd, s] permuted
    sinp = sin[perm]
    sign = np.where(np.arange(hd) % 2 == 0, np.float32(-1.0), np.float32(1.0))
    sinp = sinp * sign[:, None]
    return (
        np.ascontiguousarray(np.tile(cosp, (hpc, 1))),        # [hpc*hd, s]
        np.ascontiguousarray(np.tile(sinp, (hpc, 1))),
    )


def _master_mask(qblk):
    """master[j, u] = 1.0 iff u >= j + 384, shape [128, 384 + qblk].

    Triangle strip for any diagonal chunk is master[:, 384:512]; the c=3
    256-wide strip (zero block + triangle) is master[:, 256:512].
    """
    j = np.arange(JCH)[:, None]
    u = np.arange(384 + qblk)[None, :]
    return (u >= j + 384).astype(np.float32)


# --------------------------------------------------------------------------
# device program
# --------------------------------------------------------------------------

def emit(tc, outs, ins, *, b, s, d, mm="f32r", has_padding=False):
    import concourse.bass as bass
    import concourse.mybir as mybir

    nc = tc.nc
    f32 = mybir.dt.float32
    f32r = mybir.dt.float32r
    bf16 = mybir.dt.bfloat16
    AF = mybir.ActivationFunctionType
    mf = f32r if mm == "f32r" else f32

    bs = b * s
    kchunks = d // 128          # 8 contraction chunks for projections
    nqb = s // QBLK             # 4 query blocks per sequence
    njd = QBLK // JCH           # 4 j-chunks per query block
    ntseq = s // JCH            # 16 key chunks per sequence
    nnch = d // 128             # 8 out-proj n chunks
    scale = float(1.0 / np.sqrt(HD))

    xT, wqkv, wo = ins["xT"], ins["wqkv"], ins["wo"]
    cosT, sinTs, master, kmT = (
        ins["cosT"], ins["sinTs"], ins["master"], ins["kmT"],
    )
    yT = outs["yT"]

    # swap-adjacent-partitions shuffle mask
    SWAP_MASK = [i ^ 1 for i in range(32)]

    def sub2(ap2d, start, stride, n, w):
        """[128, n, w] AP over free columns {start + i*stride + j}."""
        sl = ap2d[:, start : start + stride * (n - 1) + w]
        return bass.AP(
            tensor=sl.tensor, offset=sl.offset,
            ap=[list(sl.ap[0])] + [[stride, n]] + [[1, w]],
        )

    import contextlib
    ctx = contextlib.ExitStack()
    with ctx:
        singles = ctx.enter_context(tc.tile_pool(name="singles", bufs=1))
        xpool = ctx.enter_context(tc.tile_pool(name="xtiles", bufs=2))
        ps_pool = ctx.enter_context(tc.tile_pool(name="ps", bufs=2, space="PSUM"))
        pav_pool = ctx.enter_context(tc.tile_pool(name="pav", bufs=2, space="PSUM"))
        pout_pool = ctx.enter_context(tc.tile_pool(name="pout", bufs=2, space="PSUM"))
        tmp_pool = ctx.enter_context(tc.tile_pool(name="tmp", bufs=2))
        qt_pool = ctx.enter_context(tc.tile_pool(name="qt", bufs=2))
        vt_pool = ctx.enter_context(tc.tile_pool(name="vt", bufs=2))
        ex_pool = ctx.enter_context(tc.tile_pool(name="ex", bufs=3))
        bct_pool = ctx.enter_context(tc.tile_pool(name="bct", bufs=2))
        bcs_pool = ctx.enter_context(tc.tile_pool(name="bcs", bufs=2))
        outh_pool = ctx.enter_context(tc.tile_pool(name="outh", bufs=2))
        ysb_pool = ctx.enter_context(tc.tile_pool(name="ysb", bufs=2))

        # ---- persistent SBUF state ----
        wqkv_sb = singles.tile([128, kchunks, 3, 128], mf)
        wo_sb = singles.tile([128, nnch, 128], mf)
        cos_sb = singles.tile([128, s], f32)
        sin_sb = singles.tile([128, s], f32)
        mst_sb = singles.tile([128, 384 + QBLK], f32)
        kT_sb = singles.tile([128, s], mf)
        # vones per sequence: h0 cols [V(0:64) | km(64)], h1 cols [km(0) | V(1:65)]
        v0_sb = singles.tile([128, ntseq, 65], mf)
        v1_sb = singles.tile([128, ntseq, 65], mf)

        nc.sync.dma_start(
            out=wqkv_sb[:, :, :, :],
            in_=wqkv.rearrange("(kc p) t n -> p kc t n", p=128),
        )

        xTr = xT.rearrange("(kc p) q -> p kc q", p=128)
        yTr = yT.rearrange("(n p) q -> p n q", p=128)

        # one iteration = emit proj+rope for block k, then attention for k-1
        nblk = b * nqb

        def emit_proj(k):
            bi, qb = divmod(k, nqb)
            g0 = bi * s + qb * QBLK
            ps0 = qb * QBLK
            t0 = qb * njd
            xt = xpool.tile([128, kchunks, QBLK], mf, tag="xt")
            if k <= 1:
                # startup: split so proj can begin after the first half lands
                nc.sync.dma_start(out=xt[:, 0:4, :], in_=xTr[:, 0:4, g0 : g0 + QBLK])
                nc.sync.dma_start(out=xt[:, 4:8, :], in_=xTr[:, 4:8, g0 : g0 + QBLK])
            else:
                nc.sync.dma_start(out=xt[:, :, :], in_=xTr[:, :, g0 : g0 + QBLK])
            if k == 0:
                # tables: after xt(0) in the SP queue (off its critical path)
                # but BEFORE any consumer is emitted
                nc.sync.dma_start(out=cos_sb[:, :], in_=cosT[:, :])
                nc.sync.dma_start(out=sin_sb[:, :], in_=sinTs[:, :])
                nc.sync.dma_start(out=mst_sb[:, :], in_=master[:, :])
                nc.sync.dma_start(
                    out=wo_sb[:, :, :], in_=wo.rearrange("c (n m) -> c n m", m=128)
                )
            if qb == 0:
                # (re)load km column of vones for this sequence (ACT queue: its
                # WAR wait on the previous batch must not block xt prefetch)
                nc.sync.dma_start(out=v0_sb[:, :, 64:65], in_=kmT[:, bi, :, :])
                nc.sync.dma_start(out=v1_sb[:, :, 64:65], in_=kmT[:, bi, :, :])
            psq = pav_pool.tile([128, QBLK], f32, tag="pav")
            psv = pout_pool.tile([128, QBLK], f32, tag="pp")
            psk = ps_pool.tile([128, 2 * QBLK], f32, tag="ps")
            for dst, ti in ((psq, 0), (psv, 2), (psk, 1)):
                for kc in range(kchunks):
                    nc.tensor.matmul(
                        dst[:, 0:QBLK] if dst is psk else dst[:, :],
                        wqkv_sb[:, kc, ti, :],
                        xt[:, kc, :],
                        start=(kc == 0),
                        stop=(kc == kchunks - 1),
                    )
            # V evacuation (cast bf16 for the 2-byte XBAR transpose); the
            # transposes + upcast copies are deferred (emitted after attn(k-1)
            # so exps lead the ACT queue)
            vt = vt_pool.tile([128, QBLK], bf16, tag="vt")
            nc.scalar.copy(out=vt[:, :], in_=psv[:, :])

            def finish():
                vtr0 = vt_pool.tile([128, njd, 64], bf16, tag="vtr0")
                vtr1 = vt_pool.tile([128, njd, 64], bf16, tag="vtr1")
                nc.scalar.dma_start_transpose(out=vtr0[:, :, :], in_=vt[0:64, :])
                nc.scalar.dma_start_transpose(out=vtr1[:, :, :], in_=vt[64:128, :])
                nc.vector.tensor_copy(v0_sb[:, t0 : t0 + njd, 0:64], vtr0[:, :, :])
                nc.vector.tensor_copy(v1_sb[:, t0 : t0 + njd, 0:64], vtr1[:, :, :])
            if has_padding:
                for c in range(njd):
                    t = t0 + c
                    km0 = bass.AP(
                        tensor=v0_sb.tensor, offset=v0_sb[:, t, 64:65].offset,
                        ap=[list(v0_sb[:, t, 64:65].ap[0])] + [[0, 65]],
                    )
                    nc.vector.tensor_mul(v0_sb[:, t, 0:65], v0_sb[:, t, 0:65], km0)
                    km1 = bass.AP(
                        tensor=v1_sb.tensor, offset=v1_sb[:, t, 64:65].offset,
                        ap=[list(v1_sb[:, t, 64:65].ap[0])] + [[0, 65]],
                    )
                    nc.vector.tensor_mul(v1_sb[:, t, 0:65], v1_sb[:, t, 0:65], km1)
            # RoPE: partners are adjacent partitions -> stream_shuffle swap.
            # q first (its rope gates the next block's scores), then k, then
            # the V upcast copies.
            tmp = tmp_pool.tile([128, 2 * QBLK], f32, tag="tmp")
            qt = qt_pool.tile([128, QBLK], mf, tag="qt")
            nc.vector.stream_shuffle(tmp[:, 0:QBLK], psq[:, 0:QBLK], SWAP_MASK)
            nc.vector.tensor_mul(tmp[:, 0:QBLK], tmp[:, 0:QBLK], sin_sb[:, ps0 : ps0 + QBLK])
            nc.vector.tensor_mul(qt[:, :], psq[:, 0:QBLK], cos_sb[:, ps0 : ps0 + QBLK])
            nc.vector.tensor_add(qt[:, :], qt[:, :], tmp[:, 0:QBLK])
            nc.vector.stream_shuffle(tmp[:, QBLK : 2 * QBLK], psk[:, 0:QBLK], SWAP_MASK)
            nc.vector.tensor_mul(
                tmp[:, QBLK : 2 * QBLK], tmp[:, QBLK : 2 * QBLK], sin_sb[:, ps0 : ps0 + QBLK]
            )
            ksl = kT_sb[:, ps0 : ps0 + QBLK]
            nc.vector.tensor_mul(ksl, psk[:, 0:QBLK], cos_sb[:, ps0 : ps0 + QBLK])
            nc.vector.tensor_add(ksl, ksl, tmp[:, QBLK : 2 * QBLK])
            return qt, finish

        def emit_attn(k, qt):
            bi, qb = divmod(k, nqb)
            g0 = bi * s + qb * QBLK
            nj = njd * (qb + 1)
            jdiag0 = njd * qb           # first diagonal chunk index
            # column subrange starts per diagonal index c. Scores write wider
            # than AV consumes (c0/c1 full width) so exp never reads stale
            # psum; AV reads only the causally-valid columns.
            DCOL_AV = (0, 128, 256, 256)
            DCOL_SC = (0, 0, 256, 256)
            pavs = []
            for h in (0, 1):
                hb = h * 64
                pav = pav_pool.tile([128, QBLK], f32, tag="pav")
                pavs.append(pav)
                r0 = 0
                vsb = v0_sb if h == 0 else v1_sb
                for pr in range(nj // 2):
                    jc0 = 2 * pr
                    E = ps_pool.tile([128, 2 * QBLK], f32, tag="ps")
                    ex = ex_pool.tile([128, 2 * QBLK], mf, tag="ex")
                    cols, sc_cols = [], []
                    for i in (0, 1):
                        jc = jc0 + i
                        c = jc - jdiag0
                        col0 = DCOL_AV[c] if c >= 0 else 0
                        sc0 = DCOL_SC[c] if c >= 0 else 0
                        cols.append(col0)
                        sc_cols.append(sc0)
                        nc.tensor.matmul(
                            E[:, i * QBLK + sc0 : (i + 1) * QBLK],
                            kT_sb[hb : hb + 64, jc * JCH : (jc + 1) * JCH],
                            qt[hb : hb + 64, sc0:QBLK],
                            start=True,
                            stop=True,
                        )
                    # exp (scale folded); subrange AP when both chunks start at 256
                    if sc_cols[0] == 256 and sc_cols[1] == 256:
                        nc.scalar.activation(
                            out=sub2(ex, 256, QBLK, 2, 256),
                            in_=sub2(E, 256, QBLK, 2, 256),
                            func=AF.Exp,
                            scale=scale,
                        )
                    else:
                        nc.scalar.activation(
                            out=ex[:, :], in_=E[:, :], func=AF.Exp, scale=scale
                        )
                    # causal masks on diagonal chunks
                    for i in (0, 1):
                        jc = jc0 + i
                        c = jc - jdiag0
                        if c >= 0:
                            if c == 3:
                                nc.vector.tensor_mul(
                                    ex[:, i * QBLK + 256 : (i + 1) * QBLK],
                                    ex[:, i * QBLK + 256 : (i + 1) * QBLK],
                                    mst_sb[:, 256:512],
                                )
                            else:
                                tc0 = i * QBLK + c * JCH
                                nc.vector.tensor_mul(
                                    ex[:, tc0 : tc0 + JCH],
                                    ex[:, tc0 : tc0 + JCH],
                                    mst_sb[:, 384:512],
                                )
                    for i in (0, 1):
                        jc = jc0 + i
                        col0 = cols[i]
                        nc.tensor.matmul(
                            pav[r0 : r0 + 65, col0:QBLK],
                            vsb[:, jc, 0:65],
                            ex[:, i * QBLK + col0 : (i + 1) * QBLK],
                            start=(jc == 0),
                            stop=(jc == nj - 1),
                            skip_group_check=True,
                        )
            # normalize: reciprocal of denominator row, 0-stride broadcast DMA, mul
            outh = outh_pool.tile([128, QBLK], mf, tag="outh")
            for h in (0, 1):
                bct = bct_pool.tile([128, QBLK], f32, tag=f"bct{h}")
                nc.vector.reciprocal(bct[64:65, :], pavs[h][64:65, :])
                row = bct[64:65, :]
                bc3 = bass.AP(
                    tensor=row.tensor, offset=row.offset,
                    ap=[list(row.ap[0])] + [[0, 64]] + [list(row.ap[1])],
                )
                nc.sync.dma_start(out=bct[0:64, :], in_=bc3)
                if h == 0:
                    nc.vector.tensor_mul(outh[0:64, :], pavs[0][0:64, :], bct[0:64, :])
                else:
                    oh1 = bcs_pool.tile([64, QBLK], mf, tag="oh1")
                    nc.vector.tensor_mul(oh1[:, :], pavs[1][0:64, :], bct[0:64, :])
                    nc.sync.dma_start(out=outh[64:128, :], in_=oh1[:, :])
            return outh

        def emit_tail(k, outh):
            bi, qb = divmod(k, nqb)
            g0 = bi * s + qb * QBLK
            # out-projection + bf16 partial writeback (one pipeline stage later
            # than attn, so the normalize chain latency hides under attention)
            ysb = ysb_pool.tile([128, nnch, QBLK], bf16, tag="ysb")
            for n in range(nnch):
                py = pout_pool.tile([128, QBLK], f32, tag="pp")
                nc.tensor.matmul(
                    py[:, :], wo_sb[:, n, :], outh[:, :], start=True, stop=True
                )
                if n % 2 == 0:
                    nc.vector.tensor_copy(ysb[:, n, :], py[:, :])
                else:
                    nc.scalar.copy(out=ysb[:, n, :], in_=py[:, :])
            nc.sync.dma_start(out=yTr[:, :, g0 : g0 + QBLK], in_=ysb[:, :, :])

        # 3-stage pipeline: iteration k emits proj(k), attn(k-1), tail(k-2), so
        # the normalize chain of attn(k-1) hides under tail(k-2) + proj(k+1).
        # At sequence boundaries proj(bi+1, 0) would overwrite kT / vones
        # chunks that attn(bi, nqb-1) still reads, so attn(k-1) goes first.
        qts, fins, ouths = {}, {}, {}
        for k in range(nblk + 2):
            boundary = k % nqb == 0

            def do_attn():
                if 1 <= k <= nblk and (k - 1) in qts:
                    ouths[k - 1] = emit_attn(k - 1, qts.pop(k - 1))

            if boundary:
                do_attn()
            if k < nblk:
                qts[k], fins[k] = emit_proj(k)
            do_attn()
            if k < nblk:
                fins.pop(k)()
            if k >= 2:
                emit_tail(k - 2, ouths.pop(k - 2))


# --------------------------------------------------------------------------
# host entry point
# --------------------------------------------------------------------------

def _shard_inputs(x, attention_mask, w_qkv, w_out, b, s, d):
    xT = np.ascontiguousarray(np.asarray(x, dtype=np.float32).reshape(b * s, d).T)
    w_qkv = np.asarray(w_qkv, dtype=np.float32)
    w_out = np.asarray(w_out, dtype=np.float32)
    cosT, sinTs = _rope_tables_T(s, HD, HPC)
    master = _master_mask(QBLK)
    am = np.asarray(attention_mask)
    # kmT[p, bi, t, 0] = mask value of key position t*128+p in sequence bi
    kmT = np.ascontiguousarray(
        (am != 0).astype(np.float32).reshape(b, s // JCH, JCH).transpose(2, 0, 1)[..., None]
    )
    perm = _perm64()
    cw = HPC * HD  # 128 columns per core
    in_maps = []
    for c in range(NCORES):
        sl = slice(c * cw, (c + 1) * cw)
        wq_c = w_qkv[:, 0 * d :][:, sl].copy()
        wk_c = w_qkv[:, 1 * d :][:, sl].copy()
        wv_c = w_qkv[:, 2 * d :][:, sl].copy()
        # permute q/k columns so rotate-half partners are adjacent partitions
        for h in range(HPC):
            blk = slice(h * HD, (h + 1) * HD)
            wq_c[:, blk] = wq_c[:, blk][:, perm]
            wk_c[:, blk] = wk_c[:, blk][:, perm]
        wqkv_c = np.ascontiguousarray(np.stack([wq_c, wk_c, wv_c], axis=1))  # [d,3,128]
        in_maps.append(
            {
                "xT": xT,
                "wqkv": wqkv_c,
                "wo": np.ascontiguousarray(w_out[sl, :]),
                "cosT": cosT,
                "sinTs": sinTs,
                "master": master,
                "kmT": kmT,
            }
        )
    return in_maps


_PROG_CACHE = {}


def _build_program(b, s, d, mm, has_padding=False):
    key = (b, s, d, mm, has_padding)
    if key in _PROG_CACHE:
        return _PROG_CACHE[key]
    import concourse.mybir as mybir
    from concourse import bacc
    from concourse.tile import TileContext

    f32 = mybir.dt.float32
    bf16 = mybir.dt.bfloat16
    mf = mybir.dt.float32r if mm == "f32r" else f32
    nc = bacc.Bacc("TRN2", target_bir_lowering=False, debug=False)
    bs = b * s
    ins = {
        "xT": nc.dram_tensor("xT", [d, bs], mf, kind="ExternalInput").ap(),
        "wqkv": nc.dram_tensor("wqkv", [d, 3, 128], mf, kind="ExternalInput").ap(),
        "wo": nc.dram_tensor("wo", [128, d], mf, kind="ExternalInput").ap(),
        "cosT": nc.dram_tensor("cosT", [128, s], f32, kind="ExternalInput").ap(),
        "sinTs": nc.dram_tensor("sinTs", [128, s], f32, kind="ExternalInput").ap(),
        "master": nc.dram_tensor("master", [128, 384 + QBLK], f32, kind="ExternalInput").ap(),
        "kmT": nc.dram_tensor("kmT", [128, b, s // JCH, 1], mf, kind="ExternalInput").ap(),
    }
    outs = {"yT": nc.dram_tensor("yT", [d, bs], bf16, kind="ExternalOutput").ap()}
    with TileContext(nc) as tc:
        emit(tc, outs, ins, b=b, s=s, d=d, mm=mm, has_padding=has_padding)
    nc.compile()
    _PROG_CACHE[key] = nc
    return nc


def kernel(x, attention_mask, w_qkv, w_out, *, mm="f32r", trace=False):
    from concourse import bass_utils

    b, s, d = x.shape
    has_padding = bool(np.any(np.asarray(attention_mask) == 0))
    nc = _build_program(b, s, d, mm, has_padding)
    in_maps = _shard_inputs(x, attention_mask, w_qkv, w_out, b, s, d)
    res = bass_utils.run_bass_kernel_spmd(
        nc, in_maps, core_ids=list(range(NCORES)), trace=trace
    )
    acc = res.results[0]["yT"].astype(np.float32)
    for c in range(1, NCORES):
        acc = acc + res.results[c]["yT"].astype(np.float32)
    out = np.ascontiguousarray(acc.T).reshape(b, s, d).astype(np.float32)
    if trace:
        return out, res
    return out


# revision 41
# speedup vs baseline: 1.1439x; 1.1439x over previous
"""Multi-head attention (B=4, S=2048, D=1024, H=16, causal + RoPE) on 8 trn2 cores.

Sharding: head-parallel. Core c owns heads {2c, 2c+1}:
  - Q/K/V projections for its 2 heads over all B*S positions,
  - RoPE + causal softmax attention,
  - row-parallel partial out-projection; host sums the 8 bf16 partials.

v2 design notes (instruction-count-driven; each matmul ~230ns fixed, each
dma_start ~625ns of serialized issue):
  - block pipeline: per (batch, 512-query-block): project q/k/v -> RoPE ->
    attention -> out-proj; block k's projection is emitted before block k-1's
    attention so PE never waits on RoPE.
  - RoPE rotate-half partners are placed on adjacent partitions by permuting
    the q/k weight columns host-side, so the half-swap is a single DVE
    stream_shuffle (mask [1,0,3,2,...]) instead of DMAs.
  - V is transposed into key-major vones layout with dma_start_transpose.
  - j-chunks processed in pairs: 2 score matmuls -> one [128,1024] exp -> 2 AV
    matmuls; diagonal chunks use column subranges; causal mask multiplies only
    the triangle strips.
  - softmax denominator via an extra vones column that carries the key-padding
    mask (1.0/0.0); the reciprocal row is broadcast across partitions with a
    K=1 matmul into PSUM instead of a broadcast DMA.
  - out partials written as bf16 (host accumulates in f32).
"""

import numpy as np

# ---- fixed problem config ----
B, S, D = 4, 2048, 1024
H, HD = 16, 64
NCORES = 8
HPC = H // NCORES          # heads per core = 2
ROPE_BASE = 10000.0

QBLK = 512                 # query block (free dim of scores/AV matmuls)
JCH = 128                  # key chunk (partition dim of scores^T)


# --------------------------------------------------------------------------
# host-side helpers
# --------------------------------------------------------------------------

def _perm64():
    """Rotate-half pairing permutation: head-dim i -> 2i, i+32 -> 2i+1."""
    p = np.empty(64, dtype=np.int64)
    p[np.arange(32) * 2] = np.arange(32)          # even slots <- dims 0..31
    p[np.arange(32) * 2 + 1] = np.arange(32, 64)  # odd slots  <- dims 32..63
    return p                                       # p[slot] = orig dim


def _rope_tables_T(s, hd, hpc):
    """cos/sin tables [hpc*hd, s] in permuted row order, sin sign-folded.

    After permutation, partition 2i holds dim i and partition 2i+1 holds dim
    i+32 (per head block of 64). rot-half swap = swap adjacent partitions;
    sign: even slots get -sin, odd slots +sin. cos/sin rows use freq of
    dim mod 32 (emb = concat(freqs, freqs)).
    """
    inv_freq = 1.0 / (ROPE_BASE ** (np.arange(0, hd, 2, dtype=np.float32) / np.float32(hd)))
    t = np.arange(s, dtype=np.float32)
    freqs = np.outer(t, inv_freq).astype(np.float32)          # [s, hd/2]
    emb = np.concatenate([freqs, freqs], axis=-1)             # [s, hd]
    cos = np.cos(emb).T.astype(np.float32)                    # [hd, s]
    sin = np.sin(emb).T.astype(np.float32)
    perm = _perm64()
    cosp = cos[perm]                                          # [hd, s] permuted
    sinp = sin[perm]
    sign = np.where(np.arange(hd) % 2 == 0, np.float32(-1.0), np.float32(1.0))
    sinp = sinp * sign[:, None]
    return (
        np.ascontiguousarray(np.tile(cosp, (hpc, 1))),        # [hpc*hd, s]
        np.ascontiguousarray(np.tile(sinp, (hpc, 1))),
    )


def _master_mask(qblk):
    """master[j, u] = 1.0 iff u >= j + 384, shape [128, 384 + qblk].

    Triangle strip for any diagonal chunk is master[:, 384:512]; the c=3
    256-wide strip (zero block + triangle) is master[:, 256:512].
    """
    j = np.arange(JCH)[:, None]
    u = np.arange(384 + qblk)[None, :]
    return (u >= j + 384).astype(np.float32)


# --------------------------------------------------------------------------
# device program
# --------------------------------------------------------------------------

def emit(tc, outs, ins, *, b, s, d, mm="f32r", has_padding=False):
    import concourse.bass as bass
    import concourse.mybir as mybir

    nc = tc.nc
    f32 = mybir.dt.float32
    f32r = mybir.dt.float32r
    bf16 = mybir.dt.bfloat16
    AF = mybir.ActivationFunctionType
    mf = f32r if mm == "f32r" else f32

    bs = b * s
    kchunks = d // 128          # 8 contraction chunks for projections
    nqb = s // QBLK             # 4 query blocks per sequence
    njd = QBLK // JCH           # 4 j-chunks per query block
    ntseq = s // JCH            # 16 key chunks per sequence
    nnch = d // 128             # 8 out-proj n chunks
    scale = float(1.0 / np.sqrt(HD))

    xT, wqkv, wo = ins["xT"], ins["wqkv"], ins["wo"]
    cosT, sinTs, master, kmT = (
        ins["cosT"], ins["sinTs"], ins["master"], ins["kmT"],
    )
    yT = outs["yT"]

    # swap-adjacent-partitions shuffle mask
    SWAP_MASK = [i ^ 1 for i in range(32)]

    def sub2(ap2d, start, stride, n, w):
        """[128, n, w] AP over free columns {start + i*stride + j}."""
        sl = ap2d[:, start : start + stride * (n - 1) + w]
        return bass.AP(
            tensor=sl.tensor, offset=sl.offset,
            ap=[list(sl.ap[0])] + [[stride, n]] + [[1, w]],
        )

    import contextlib
    ctx = contextlib.ExitStack()
    with ctx:
        singles = ctx.enter_context(tc.tile_pool(name="singles", bufs=1))
        xpool = ctx.enter_context(tc.tile_pool(name="xtiles", bufs=2))
        ps_pool = ctx.enter_context(tc.tile_pool(name="ps", bufs=2, space="PSUM"))
        pav_pool = ctx.enter_context(tc.tile_pool(name="pav", bufs=2, space="PSUM"))
        pout_pool = ctx.enter_context(tc.tile_pool(name="pout", bufs=2, space="PSUM"))
        tmp_pool = ctx.enter_context(tc.tile_pool(name="tmp", bufs=2))
        qt_pool = ctx.enter_context(tc.tile_pool(name="qt", bufs=2))
        vt_pool = ctx.enter_context(tc.tile_pool(name="vt", bufs=2))
        vtr_pool = ctx.enter_context(tc.tile_pool(name="vtr", bufs=4))
        ex_pool = ctx.enter_context(tc.tile_pool(name="ex", bufs=3))
        bct_pool = ctx.enter_context(tc.tile_pool(name="bct", bufs=2))
        bcs_pool = ctx.enter_context(tc.tile_pool(name="bcs", bufs=2))
        outh_pool = ctx.enter_context(tc.tile_pool(name="outh", bufs=2))
        ysb_pool = ctx.enter_context(tc.tile_pool(name="ysb", bufs=2))

        # ---- persistent SBUF state ----
        wqkv_sb = singles.tile([128, kchunks, 3, 128], mf)
        wo_sb = singles.tile([128, nnch, 128], mf)
        cos_sb = singles.tile([128, s], f32)
        sin_sb = singles.tile([128, s], f32)
        mst_sb = singles.tile([128, 384 + QBLK], f32)
        kT_sb = singles.tile([128, s], mf)
        # vones per sequence: h0 cols [V(0:64) | km(64)], h1 cols [km(0) | V(1:65)]
        v0_sb = singles.tile([128, ntseq, 65], mf)
        v1_sb = singles.tile([128, ntseq, 65], mf)

        nc.sync.dma_start(
            out=wqkv_sb[:, :, :, :],
            in_=wqkv.rearrange("(kc p) t n -> p kc t n", p=128),
        )

        xTr = xT.rearrange("(kc p) q -> p kc q", p=128)
        yTr = yT.rearrange("(n p) q -> p n q", p=128)

        # one iteration = emit proj+rope for block k, then attention for k-1
        nblk = b * nqb

        def emit_proj(k):
            bi, qb = divmod(k, nqb)
            g0 = bi * s + qb * QBLK
            ps0 = qb * QBLK
            t0 = qb * njd
            xt = xpool.tile([128, kchunks, QBLK], mf, tag="xt")
            if k <= 1:
                # startup: split so proj can begin after the first half lands
                nc.sync.dma_start(out=xt[:, 0:4, :], in_=xTr[:, 0:4, g0 : g0 + QBLK])
                nc.sync.dma_start(out=xt[:, 4:8, :], in_=xTr[:, 4:8, g0 : g0 + QBLK])
            else:
                nc.sync.dma_start(out=xt[:, :, :], in_=xTr[:, :, g0 : g0 + QBLK])
            if k == 0:
                # tables: after xt(0) in the SP queue (off its critical path)
                # but BEFORE any consumer is emitted
                nc.sync.dma_start(out=cos_sb[:, :], in_=cosT[:, :])
                nc.sync.dma_start(out=sin_sb[:, :], in_=sinTs[:, :])
            if qb == 0:
                # (re)load km column of vones for this sequence (ACT queue: its
                # WAR wait on the previous batch must not block xt prefetch)
                nc.sync.dma_start(out=v0_sb[:, :, 64:65], in_=kmT[:, bi, :, :])
                nc.sync.dma_start(out=v1_sb[:, :, 64:65], in_=kmT[:, bi, :, :])
            psq = pav_pool.tile([128, QBLK], f32, tag="pav")
            psv = pout_pool.tile([128, QBLK], f32, tag="pp")
            psk = ps_pool.tile([128, 2 * QBLK], f32, tag="ps")
            for dst, ti in ((psq, 0), (psv, 2), (psk, 1)):
                for kc in range(kchunks):
                    nc.tensor.matmul(
                        dst[:, 0:QBLK] if dst is psk else dst[:, :],
                        wqkv_sb[:, kc, ti, :],
                        xt[:, kc, :],
                        start=(kc == 0),
                        stop=(kc == kchunks - 1),
                    )
            # V evacuation (cast bf16 for the 2-byte XBAR transpose); the
            # transposes + upcast copies are deferred (emitted after attn(k-1)
            # so exps lead the ACT queue)
            vt = vt_pool.tile([128, QBLK], bf16, tag="vt")
            nc.vector.tensor_copy(vt[:, :], psv[:, :])

            def finish():
                vtr0 = vtr_pool.tile([128, njd, 64], bf16, tag="vtr0")
                vtr1 = vtr_pool.tile([128, njd, 64], bf16, tag="vtr1")
                nc.scalar.dma_start_transpose(out=vtr0[:, :, :], in_=vt[0:64, :])
                nc.scalar.dma_start_transpose(out=vtr1[:, :, :], in_=vt[64:128, :])
                nc.scalar.copy(out=v0_sb[:, t0 : t0 + njd, 0:64], in_=vtr0[:, :, :])
                nc.scalar.copy(out=v1_sb[:, t0 : t0 + njd, 0:64], in_=vtr1[:, :, :])
            if has_padding:
                for c in range(njd):
                    t = t0 + c
                    km0 = bass.AP(
                        tensor=v0_sb.tensor, offset=v0_sb[:, t, 64:65].offset,
                        ap=[list(v0_sb[:, t, 64:65].ap[0])] + [[0, 65]],
                    )
                    nc.vector.tensor_mul(v0_sb[:, t, 0:65], v0_sb[:, t, 0:65], km0)
                    km1 = bass.AP(
                        tensor=v1_sb.tensor, offset=v1_sb[:, t, 64:65].offset,
                        ap=[list(v1_sb[:, t, 64:65].ap[0])] + [[0, 65]],
                    )
                    nc.vector.tensor_mul(v1_sb[:, t, 0:65], v1_sb[:, t, 0:65], km1)
            # RoPE: partners are adjacent partitions -> stream_shuffle swap.
            # q first (its rope gates the next block's scores), then k, then
            # the V upcast copies.
            # k first: the next attn's scores recycle psk's psum buf, so
            # rope-k is on the critical path; rope-q isn't needed until the
            # NEXT iteration's attention.
            tmp = tmp_pool.tile([128, 2 * QBLK], f32, tag="tmp")
            qt = qt_pool.tile([128, QBLK], mf, tag="qt")
            nc.vector.stream_shuffle(tmp[:, QBLK : 2 * QBLK], psk[:, 0:QBLK], SWAP_MASK)
            nc.vector.tensor_mul(
                tmp[:, QBLK : 2 * QBLK], tmp[:, QBLK : 2 * QBLK], sin_sb[:, ps0 : ps0 + QBLK]
            )
            ksl = kT_sb[:, ps0 : ps0 + QBLK]
            nc.vector.tensor_mul(ksl, psk[:, 0:QBLK], cos_sb[:, ps0 : ps0 + QBLK])
            nc.vector.tensor_add(ksl, ksl, tmp[:, QBLK : 2 * QBLK])
            nc.vector.stream_shuffle(tmp[:, 0:QBLK], psq[:, 0:QBLK], SWAP_MASK)
            nc.vector.tensor_mul(tmp[:, 0:QBLK], tmp[:, 0:QBLK], sin_sb[:, ps0 : ps0 + QBLK])
            nc.vector.tensor_mul(qt[:, :], psq[:, 0:QBLK], cos_sb[:, ps0 : ps0 + QBLK])
            nc.vector.tensor_add(qt[:, :], qt[:, :], tmp[:, 0:QBLK])
            return qt, finish

        def emit_attn(k, qt):
            if k == 0:
                nc.sync.dma_start(out=mst_sb[:, :], in_=master[:, :])
            bi, qb = divmod(k, nqb)
            g0 = bi * s + qb * QBLK
            nj = njd * (qb + 1)
            jdiag0 = njd * qb           # first diagonal chunk index
            # column subrange starts per diagonal index c. Scores write wider
            # than AV consumes (c0/c1 full width) so exp never reads stale
            # psum; AV reads only the causally-valid columns.
            DCOL_AV = (0, 128, 256, 256)
            DCOL_SC = (0, 0, 256, 256)
            pavs = []
            for h in (0, 1):
                hb = h * 64
                pav = pav_pool.tile([128, QBLK], f32, tag="pav")
                pavs.append(pav)
                r0 = 0
                vsb = v0_sb if h == 0 else v1_sb
                for pr in range(nj // 2):
                    jc0 = 2 * pr
                    E = ps_pool.tile([128, 2 * QBLK], f32, tag="ps")
                    ex = ex_pool.tile([128, 2 * QBLK], mf, tag="ex")
                    cols, sc_cols = [], []
                    for i in (0, 1):
                        jc = jc0 + i
                        c = jc - jdiag0
                        col0 = DCOL_AV[c] if c >= 0 else 0
                        sc0 = DCOL_SC[c] if c >= 0 else 0
                        cols.append(col0)
                        sc_cols.append(sc0)
                        nc.tensor.matmul(
                            E[:, i * QBLK + sc0 : (i + 1) * QBLK],
                            kT_sb[hb : hb + 64, jc * JCH : (jc + 1) * JCH],
                            qt[hb : hb + 64, sc0:QBLK],
                            start=True,
                            stop=True,
                        )
                    # exp (scale folded); subrange AP when both chunks start at 256
                    if sc_cols[0] == 256 and sc_cols[1] == 256:
                        nc.scalar.activation(
                            out=sub2(ex, 256, QBLK, 2, 256),
                            in_=sub2(E, 256, QBLK, 2, 256),
                            func=AF.Exp,
                            scale=scale,
                        )
                    else:
                        nc.scalar.activation(
                            out=ex[:, :], in_=E[:, :], func=AF.Exp, scale=scale
                        )
                    # causal masks on diagonal chunks
                    for i in (0, 1):
                        jc = jc0 + i
                        c = jc - jdiag0
                        if c >= 0:
                            if c == 3:
                                nc.vector.tensor_mul(
                                    ex[:, i * QBLK + 256 : (i + 1) * QBLK],
                                    ex[:, i * QBLK + 256 : (i + 1) * QBLK],
                                    mst_sb[:, 256:512],
                                )
                            else:
                                tc0 = i * QBLK + c * JCH
                                nc.vector.tensor_mul(
                                    ex[:, tc0 : tc0 + JCH],
                                    ex[:, tc0 : tc0 + JCH],
                                    mst_sb[:, 384:512],
                                )
                    for i in (0, 1):
                        jc = jc0 + i
                        col0 = cols[i]
                        nc.tensor.matmul(
                            pav[r0 : r0 + 65, col0:QBLK],
                            vsb[:, jc, 0:65],
                            ex[:, i * QBLK + col0 : (i + 1) * QBLK],
                            start=(jc == 0),
                            stop=(jc == nj - 1),
                            skip_group_check=True,
                        )
            # normalize: reciprocal of denominator row, 0-stride broadcast DMA, mul
            outh = outh_pool.tile([128, QBLK], mf, tag="outh")
            for h in (0, 1):
                bct = bct_pool.tile([128, QBLK], f32, tag=f"bct{h}")
                nc.vector.reciprocal(bct[64:65, :], pavs[h][64:65, :])
                row = bct[64:65, :]
                bc3 = bass.AP(
                    tensor=row.tensor, offset=row.offset,
                    ap=[list(row.ap[0])] + [[0, 64]] + [list(row.ap[1])],
                )
                nc.sync.dma_start(out=bct[0:64, :], in_=bc3)
                if h == 0:
                    nc.vector.tensor_mul(outh[0:64, :], pavs[0][0:64, :], bct[0:64, :])
                else:
                    oh1 = bcs_pool.tile([64, QBLK], mf, tag="oh1")
                    nc.vector.tensor_mul(oh1[:, :], pavs[1][0:64, :], bct[0:64, :])
                    nc.sync.dma_start(out=outh[64:128, :], in_=oh1[:, :])
            return outh

        def emit_tail(k, outh):
            if k == 0:
                nc.sync.dma_start(
                    out=wo_sb[:, :, :], in_=wo.rearrange("c (n m) -> c n m", m=128)
                )
            bi, qb = divmod(k, nqb)
            g0 = bi * s + qb * QBLK
            # out-projection + bf16 partial writeback (one pipeline stage later
            # than attn, so the normalize chain latency hides under attention)
            ysb = ysb_pool.tile([128, nnch, QBLK], bf16, tag="ysb")
            for n in range(nnch):
                py = pout_pool.tile([128, QBLK], f32, tag="pp")
                nc.tensor.matmul(
                    py[:, :], wo_sb[:, n, :], outh[:, :], start=True, stop=True
                )
                if n % 2 == 0:
                    nc.vector.tensor_copy(ysb[:, n, :], py[:, :])
                else:
                    nc.scalar.copy(out=ysb[:, n, :], in_=py[:, :])
            nc.sync.dma_start(out=yTr[:, :, g0 : g0 + QBLK], in_=ysb[:, :, :])

        # 3-stage pipeline: iteration k emits proj(k), attn(k-1), tail(k-2), so
        # the normalize chain of attn(k-1) hides under tail(k-2) + proj(k+1).
        # At sequence boundaries proj(bi+1, 0) would overwrite kT / vones
        # chunks that attn(bi, nqb-1) still reads, so attn(k-1) goes first.
        qts, fins, ouths = {}, {}, {}
        for k in range(nblk + 2):
            boundary = k % nqb == 0

            def do_attn():
                if 1 <= k <= nblk and (k - 1) in qts:
                    ouths[k - 1] = emit_attn(k - 1, qts.pop(k - 1))

            if boundary:
                do_attn()
            if k < nblk:
                qts[k], fins[k] = emit_proj(k)
            do_attn()
            if k < nblk:
                fins.pop(k)()
            if k >= 2:
                emit_tail(k - 2, ouths.pop(k - 2))


# --------------------------------------------------------------------------
# host entry point
# --------------------------------------------------------------------------

def _shard_inputs(x, attention_mask, w_qkv, w_out, b, s, d):
    xT = np.ascontiguousarray(np.asarray(x, dtype=np.float32).reshape(b * s, d).T)
    w_qkv = np.asarray(w_qkv, dtype=np.float32)
    w_out = np.asarray(w_out, dtype=np.float32)
    cosT, sinTs = _rope_tables_T(s, HD, HPC)
    master = _master_mask(QBLK)
    am = np.asarray(attention_mask)
    # kmT[p, bi, t, 0] = mask value of key position t*128+p in sequence bi
    kmT = np.ascontiguousarray(
        (am != 0).astype(np.float32).reshape(b, s // JCH, JCH).transpose(2, 0, 1)[..., None]
    )
    perm = _perm64()
    cw = HPC * HD  # 128 columns per core
    in_maps = []
    for c in range(NCORES):
        sl = slice(c * cw, (c + 1) * cw)
        wq_c = w_qkv[:, 0 * d :][:, sl].copy()
        wk_c = w_qkv[:, 1 * d :][:, sl].copy()
        wv_c = w_qkv[:, 2 * d :][:, sl].copy()
        # permute q/k columns so rotate-half partners are adjacent partitions
        for h in range(HPC):
            blk = slice(h * HD, (h + 1) * HD)
            wq_c[:, blk] = wq_c[:, blk][:, perm]
            wk_c[:, blk] = wk_c[:, blk][:, perm]
        wqkv_c = np.ascontiguousarray(np.stack([wq_c, wk_c, wv_c], axis=1))  # [d,3,128]
        in_maps.append(
            {
                "xT": xT,
                "wqkv": wqkv_c,
                "wo": np.ascontiguousarray(w_out[sl, :]),
                "cosT": cosT,
                "sinTs": sinTs,
                "master": master,
                "kmT": kmT,
            }
        )
    return in_maps


_PROG_CACHE = {}


def _build_program(b, s, d, mm, has_padding=False):
    key = (b, s, d, mm, has_padding)
    if key in _PROG_CACHE:
        return _PROG_CACHE[key]
    import concourse.mybir as mybir
    from concourse import bacc
    from concourse.tile import TileContext

    f32 = mybir.dt.float32
    bf16 = mybir.dt.bfloat16
    mf = mybir.dt.float32r if mm == "f32r" else f32
    nc = bacc.Bacc("TRN2", target_bir_lowering=False, debug=False)
    bs = b * s
    ins = {
        "xT": nc.dram_tensor("xT", [d, bs], mf, kind="ExternalInput").ap(),
        "wqkv": nc.dram_tensor("wqkv", [d, 3, 128], mf, kind="ExternalInput").ap(),
        "wo": nc.dram_tensor("wo", [128, d], mf, kind="ExternalInput").ap(),
        "cosT": nc.dram_tensor("cosT", [128, s], f32, kind="ExternalInput").ap(),
        "sinTs": nc.dram_tensor("sinTs", [128, s], f32, kind="ExternalInput").ap(),
        "master": nc.dram_tensor("master", [128, 384 + QBLK], f32, kind="ExternalInput").ap(),
        "kmT": nc.dram_tensor("kmT", [128, b, s // JCH, 1], mf, kind="ExternalInput").ap(),
    }
    outs = {"yT": nc.dram_tensor("yT", [d, bs], bf16, kind="ExternalOutput").ap()}
    with TileContext(nc) as tc:
        emit(tc, outs, ins, b=b, s=s, d=d, mm=mm, has_padding=has_padding)
    nc.compile()
    _PROG_CACHE[key] = nc
    return nc


def kernel(x, attention_mask, w_qkv, w_out, *, mm="f32r", trace=False):
    from concourse import bass_utils

    b, s, d = x.shape
    has_padding = bool(np.any(np.asarray(attention_mask) == 0))
    nc = _build_program(b, s, d, mm, has_padding)
    in_maps = _shard_inputs(x, attention_mask, w_qkv, w_out, b, s, d)
    res = bass_utils.run_bass_kernel_spmd(
        nc, in_maps, core_ids=list(range(NCORES)), trace=trace
    )
    acc = res.results[0]["yT"].astype(np.float32)
    for c in range(1, NCORES):
        acc = acc + res.results[c]["yT"].astype(np.float32)
    out = np.ascontiguousarray(acc.T).reshape(b, s, d).astype(np.float32)
    if trace:
        return out, res
    return out


# revision 42
# speedup vs baseline: 1.2600x; 1.1015x over previous
"""Multi-head attention (B=4, S=2048, D=1024, H=16, causal + RoPE) on 8 trn2 cores.

Sharding: head-parallel. Core c owns heads {2c, 2c+1}:
  - Q/K/V projections for its 2 heads over all B*S positions,
  - RoPE + causal softmax attention,
  - row-parallel partial out-projection; host sums the 8 bf16 partials.

v2 design notes (instruction-count-driven; each matmul ~230ns fixed, each
dma_start ~625ns of serialized issue):
  - block pipeline: per (batch, 512-query-block): project q/k/v -> RoPE ->
    attention -> out-proj; block k's projection is emitted before block k-1's
    attention so PE never waits on RoPE.
  - RoPE rotate-half partners are placed on adjacent partitions by permuting
    the q/k weight columns host-side, so the half-swap is a single DVE
    stream_shuffle (mask [1,0,3,2,...]) instead of DMAs.
  - V is transposed into key-major vones layout with dma_start_transpose.
  - j-chunks processed in pairs: 2 score matmuls -> one [128,1024] exp -> 2 AV
    matmuls; diagonal chunks use column subranges; causal mask multiplies only
    the triangle strips.
  - softmax denominator via an extra vones column that carries the key-padding
    mask (1.0/0.0); the reciprocal row is broadcast across partitions with a
    K=1 matmul into PSUM instead of a broadcast DMA.
  - out partials written as bf16 (host accumulates in f32).
"""

import numpy as np

# ---- fixed problem config ----
B, S, D = 4, 2048, 1024
H, HD = 16, 64
NCORES = 8
HPC = H // NCORES          # heads per core = 2
ROPE_BASE = 10000.0

QBLK = 512                 # query block (free dim of scores/AV matmuls)
JCH = 128                  # key chunk (partition dim of scores^T)


# --------------------------------------------------------------------------
# host-side helpers
# --------------------------------------------------------------------------

def _perm64():
    """Rotate-half pairing permutation: head-dim i -> 2i, i+32 -> 2i+1."""
    p = np.empty(64, dtype=np.int64)
    p[np.arange(32) * 2] = np.arange(32)          # even slots <- dims 0..31
    p[np.arange(32) * 2 + 1] = np.arange(32, 64)  # odd slots  <- dims 32..63
    return p                                       # p[slot] = orig dim


def _rope_tables_T(s, hd, hpc):
    """cos/sin tables [hpc*hd, s] in permuted row order, sin sign-folded.

    After permutation, partition 2i holds dim i and partition 2i+1 holds dim
    i+32 (per head block of 64). rot-half swap = swap adjacent partitions;
    sign: even slots get -sin, odd slots +sin. cos/sin rows use freq of
    dim mod 32 (emb = concat(freqs, freqs)).
    """
    inv_freq = 1.0 / (ROPE_BASE ** (np.arange(0, hd, 2, dtype=np.float32) / np.float32(hd)))
    t = np.arange(s, dtype=np.float32)
    freqs = np.outer(t, inv_freq).astype(np.float32)          # [s, hd/2]
    emb = np.concatenate([freqs, freqs], axis=-1)             # [s, hd]
    cos = np.cos(emb).T.astype(np.float32)                    # [hd, s]
    sin = np.sin(emb).T.astype(np.float32)
    perm = _perm64()
    cosp = cos[perm]                                          # [hd, s] permuted
    sinp = sin[perm]
    sign = np.where(np.arange(hd) % 2 == 0, np.float32(-1.0), np.float32(1.0))
    sinp = sinp * sign[:, None]
    return (
        np.ascontiguousarray(np.tile(cosp, (hpc, 1))),        # [hpc*hd, s]
        np.ascontiguousarray(np.tile(sinp, (hpc, 1))),
    )


def _master_mask(qblk):
    """master[j, u] = 1.0 iff u >= j + 384, shape [128, 384 + qblk].

    Triangle strip for any diagonal chunk is master[:, 384:512]; the c=3
    256-wide strip (zero block + triangle) is master[:, 256:512].
    """
    j = np.arange(JCH)[:, None]
    u = np.arange(384 + qblk)[None, :]
    return (u >= j + 384).astype(np.float32)


# --------------------------------------------------------------------------
# device program
# --------------------------------------------------------------------------

def emit(tc, outs, ins, *, b, s, d, mm="f32r", has_padding=False):
    import concourse.bass as bass
    import concourse.mybir as mybir

    nc = tc.nc
    f32 = mybir.dt.float32
    f32r = mybir.dt.float32r
    bf16 = mybir.dt.bfloat16
    AF = mybir.ActivationFunctionType
    mf = f32r if mm == "f32r" else f32

    bs = b * s
    kchunks = d // 128          # 8 contraction chunks for projections
    nqb = s // QBLK             # 4 query blocks per sequence
    njd = QBLK // JCH           # 4 j-chunks per query block
    ntseq = s // JCH            # 16 key chunks per sequence
    nnch = d // 128             # 8 out-proj n chunks
    scale = float(1.0 / np.sqrt(HD))

    xT, wqkv, wo = ins["xT"], ins["wqkv"], ins["wo"]
    cosT, sinTs, master, kmT = (
        ins["cosT"], ins["sinTs"], ins["master"], ins["kmT"],
    )
    yT = outs["yT"]

    # swap-adjacent-partitions shuffle mask
    SWAP_MASK = [i ^ 1 for i in range(32)]

    def sub2(ap2d, start, stride, n, w):
        """[128, n, w] AP over free columns {start + i*stride + j}."""
        sl = ap2d[:, start : start + stride * (n - 1) + w]
        return bass.AP(
            tensor=sl.tensor, offset=sl.offset,
            ap=[list(sl.ap[0])] + [[stride, n]] + [[1, w]],
        )

    import contextlib
    ctx = contextlib.ExitStack()
    with ctx:
        singles = ctx.enter_context(tc.tile_pool(name="singles", bufs=1))
        xpool = ctx.enter_context(tc.tile_pool(name="xtiles", bufs=2))
        ps_pool = ctx.enter_context(tc.tile_pool(name="ps", bufs=2, space="PSUM"))
        pav_pool = ctx.enter_context(tc.tile_pool(name="pav", bufs=2, space="PSUM"))
        pout_pool = ctx.enter_context(tc.tile_pool(name="pout", bufs=2, space="PSUM"))
        tmp_pool = ctx.enter_context(tc.tile_pool(name="tmp", bufs=2))
        qt_pool = ctx.enter_context(tc.tile_pool(name="qt", bufs=2))
        vt_pool = ctx.enter_context(tc.tile_pool(name="vt", bufs=2))
        vtr_pool = ctx.enter_context(tc.tile_pool(name="vtr", bufs=4))
        ex_pool = ctx.enter_context(tc.tile_pool(name="ex", bufs=3))
        bct_pool = ctx.enter_context(tc.tile_pool(name="bct", bufs=2))
        bcs_pool = ctx.enter_context(tc.tile_pool(name="bcs", bufs=2))
        outh_pool = ctx.enter_context(tc.tile_pool(name="outh", bufs=2))
        ysb_pool = ctx.enter_context(tc.tile_pool(name="ysb", bufs=2))

        # ---- persistent SBUF state ----
        wqkv_sb = singles.tile([128, kchunks, 3, 128], mf)
        wo_sb = singles.tile([128, nnch, 128], mf)
        cos_sb = singles.tile([128, s], f32)
        sin_sb = singles.tile([128, s], f32)
        mst_sb = singles.tile([128, 384 + QBLK], f32)
        kT_sb = singles.tile([128, s], mf)
        # vones per sequence: h0 cols [V(0:64) | km(64)], h1 cols [km(0) | V(1:65)]
        v0_sb = singles.tile([128, ntseq, 65], mf)
        v1_sb = singles.tile([128, ntseq, 65], mf)

        nc.sync.dma_start(
            out=wqkv_sb[:, :, :, :],
            in_=wqkv.rearrange("(kc p) t n -> p kc t n", p=128),
        )

        xTr = xT.rearrange("(kc p) q -> p kc q", p=128)
        yTr = yT.rearrange("(n p) q -> p n q", p=128)

        # one iteration = emit proj+rope for block k, then attention for k-1
        nblk = b * nqb

        def emit_proj(k):
            bi, qb = divmod(k, nqb)
            g0 = bi * s + qb * QBLK
            ps0 = qb * QBLK
            t0 = qb * njd
            xt = xpool.tile([128, kchunks, QBLK], mf, tag="xt")
            if k <= 1:
                # startup: split so proj can begin after the first half lands
                nc.sync.dma_start(out=xt[:, 0:4, :], in_=xTr[:, 0:4, g0 : g0 + QBLK])
                nc.sync.dma_start(out=xt[:, 4:8, :], in_=xTr[:, 4:8, g0 : g0 + QBLK])
            else:
                nc.sync.dma_start(out=xt[:, :, :], in_=xTr[:, :, g0 : g0 + QBLK])
            if k == 0:
                # tables: after xt(0) in the SP queue (off its critical path)
                # but BEFORE any consumer is emitted
                nc.sync.dma_start(out=cos_sb[:, :], in_=cosT[:, :])
                nc.sync.dma_start(out=sin_sb[:, :], in_=sinTs[:, :])
            if qb == 0:
                # (re)load km column of vones for this sequence (ACT queue: its
                # WAR wait on the previous batch must not block xt prefetch)
                nc.sync.dma_start(out=v0_sb[:, :, 64:65], in_=kmT[:, bi, :, :])
                nc.sync.dma_start(out=v1_sb[:, :, 64:65], in_=kmT[:, bi, :, :])
            psq = pout_pool.tile([128, QBLK], f32, tag="pp")
            psv = pout_pool.tile([128, QBLK], f32, tag="pp")
            psk = ps_pool.tile([128, 2 * QBLK], f32, tag="ps")
            for dst, ti in ((psq, 0), (psv, 2), (psk, 1)):
                for kc in range(kchunks):
                    nc.tensor.matmul(
                        dst[:, 0:QBLK] if dst is psk else dst[:, :],
                        wqkv_sb[:, kc, ti, :],
                        xt[:, kc, :],
                        start=(kc == 0),
                        stop=(kc == kchunks - 1),
                    )
            # V evacuation (cast bf16 for the 2-byte XBAR transpose); the
            # transposes + upcast copies are deferred (emitted after attn(k-1)
            # so exps lead the ACT queue)
            vt = vt_pool.tile([128, QBLK], bf16, tag="vt")
            nc.vector.tensor_copy(vt[:, :], psv[:, :])

            def finish():
                vtr0 = vtr_pool.tile([128, njd, 64], bf16, tag="vtr0")
                vtr1 = vtr_pool.tile([128, njd, 64], bf16, tag="vtr1")
                nc.scalar.dma_start_transpose(out=vtr0[:, :, :], in_=vt[0:64, :])
                nc.scalar.dma_start_transpose(out=vtr1[:, :, :], in_=vt[64:128, :])
                nc.scalar.copy(out=v0_sb[:, t0 : t0 + njd, 0:64], in_=vtr0[:, :, :])
                nc.scalar.copy(out=v1_sb[:, t0 : t0 + njd, 0:64], in_=vtr1[:, :, :])
            if has_padding:
                for c in range(njd):
                    t = t0 + c
                    km0 = bass.AP(
                        tensor=v0_sb.tensor, offset=v0_sb[:, t, 64:65].offset,
                        ap=[list(v0_sb[:, t, 64:65].ap[0])] + [[0, 65]],
                    )
                    nc.vector.tensor_mul(v0_sb[:, t, 0:65], v0_sb[:, t, 0:65], km0)
                    km1 = bass.AP(
                        tensor=v1_sb.tensor, offset=v1_sb[:, t, 64:65].offset,
                        ap=[list(v1_sb[:, t, 64:65].ap[0])] + [[0, 65]],
                    )
                    nc.vector.tensor_mul(v1_sb[:, t, 0:65], v1_sb[:, t, 0:65], km1)
            # RoPE: partners are adjacent partitions -> stream_shuffle swap.
            # q first (its rope gates the next block's scores), then k, then
            # the V upcast copies.
            # k first: the next attn's scores recycle psk's psum buf, so
            # rope-k is on the critical path; rope-q isn't needed until the
            # NEXT iteration's attention.
            tmp = tmp_pool.tile([128, 2 * QBLK], f32, tag="tmp")
            qt = qt_pool.tile([128, QBLK], mf, tag="qt")
            nc.vector.stream_shuffle(tmp[:, QBLK : 2 * QBLK], psk[:, 0:QBLK], SWAP_MASK)
            nc.vector.tensor_mul(
                tmp[:, QBLK : 2 * QBLK], tmp[:, QBLK : 2 * QBLK], sin_sb[:, ps0 : ps0 + QBLK]
            )
            ksl = kT_sb[:, ps0 : ps0 + QBLK]
            nc.vector.tensor_mul(ksl, psk[:, 0:QBLK], cos_sb[:, ps0 : ps0 + QBLK])
            nc.vector.tensor_add(ksl, ksl, tmp[:, QBLK : 2 * QBLK])
            nc.vector.stream_shuffle(tmp[:, 0:QBLK], psq[:, 0:QBLK], SWAP_MASK)
            nc.vector.tensor_mul(tmp[:, 0:QBLK], tmp[:, 0:QBLK], sin_sb[:, ps0 : ps0 + QBLK])
            nc.vector.tensor_mul(qt[:, :], psq[:, 0:QBLK], cos_sb[:, ps0 : ps0 + QBLK])
            nc.vector.tensor_add(qt[:, :], qt[:, :], tmp[:, 0:QBLK])
            return qt, finish

        def emit_attn(k, qt):
            if k == 0:
                nc.sync.dma_start(out=mst_sb[:, :], in_=master[:, :])
            bi, qb = divmod(k, nqb)
            g0 = bi * s + qb * QBLK
            nj = njd * (qb + 1)
            jdiag0 = njd * qb           # first diagonal chunk index
            # column subrange starts per diagonal index c. Scores write wider
            # than AV consumes (c0/c1 full width) so exp never reads stale
            # psum; AV reads only the causally-valid columns.
            DCOL_AV = (0, 128, 256, 256)
            DCOL_SC = (0, 0, 256, 256)
            pavs = []
            for h in (0, 1):
                hb = h * 64
                pav = pav_pool.tile([128, QBLK], f32, tag="pav")
                pavs.append(pav)
                r0 = 0
                vsb = v0_sb if h == 0 else v1_sb
                for pr in range(nj // 2):
                    jc0 = 2 * pr
                    E = ps_pool.tile([128, 2 * QBLK], f32, tag="ps")
                    ex = ex_pool.tile([128, 2 * QBLK], mf, tag="ex")
                    cols, sc_cols = [], []
                    for i in (0, 1):
                        jc = jc0 + i
                        c = jc - jdiag0
                        col0 = DCOL_AV[c] if c >= 0 else 0
                        sc0 = DCOL_SC[c] if c >= 0 else 0
                        cols.append(col0)
                        sc_cols.append(sc0)
                        nc.tensor.matmul(
                            E[:, i * QBLK + sc0 : (i + 1) * QBLK],
                            kT_sb[hb : hb + 64, jc * JCH : (jc + 1) * JCH],
                            qt[hb : hb + 64, sc0:QBLK],
                            start=True,
                            stop=True,
                        )
                    # exp (scale folded); subrange AP when both chunks start at 256
                    if sc_cols[0] == 256 and sc_cols[1] == 256:
                        nc.scalar.activation(
                            out=sub2(ex, 256, QBLK, 2, 256),
                            in_=sub2(E, 256, QBLK, 2, 256),
                            func=AF.Exp,
                            scale=scale,
                        )
                    else:
                        nc.scalar.activation(
                            out=ex[:, :], in_=E[:, :], func=AF.Exp, scale=scale
                        )
                    # causal masks on diagonal chunks
                    for i in (0, 1):
                        jc = jc0 + i
                        c = jc - jdiag0
                        if c >= 0:
                            if c == 3:
                                nc.vector.tensor_mul(
                                    ex[:, i * QBLK + 256 : (i + 1) * QBLK],
                                    ex[:, i * QBLK + 256 : (i + 1) * QBLK],
                                    mst_sb[:, 256:512],
                                )
                            else:
                                tc0 = i * QBLK + c * JCH
                                nc.vector.tensor_mul(
                                    ex[:, tc0 : tc0 + JCH],
                                    ex[:, tc0 : tc0 + JCH],
                                    mst_sb[:, 384:512],
                                )
                    for i in (0, 1):
                        jc = jc0 + i
                        col0 = cols[i]
                        nc.tensor.matmul(
                            pav[r0 : r0 + 65, col0:QBLK],
                            vsb[:, jc, 0:65],
                            ex[:, i * QBLK + col0 : (i + 1) * QBLK],
                            start=(jc == 0),
                            stop=(jc == nj - 1),
                            skip_group_check=True,
                        )
            # normalize: reciprocal of denominator row, 0-stride broadcast DMA, mul
            outh = outh_pool.tile([128, QBLK], mf, tag="outh")
            for h in (0, 1):
                bct = bct_pool.tile([128, QBLK], f32, tag=f"bct{h}")
                nc.vector.reciprocal(bct[64:65, :], pavs[h][64:65, :])
                row = bct[64:65, :]
                bc3 = bass.AP(
                    tensor=row.tensor, offset=row.offset,
                    ap=[list(row.ap[0])] + [[0, 64]] + [list(row.ap[1])],
                )
                nc.sync.dma_start(out=bct[0:64, :], in_=bc3)
                if h == 0:
                    nc.vector.tensor_mul(outh[0:64, :], pavs[0][0:64, :], bct[0:64, :])
                else:
                    oh1 = bcs_pool.tile([64, QBLK], mf, tag="oh1")
                    nc.vector.tensor_mul(oh1[:, :], pavs[1][0:64, :], bct[0:64, :])
                    nc.sync.dma_start(out=outh[64:128, :], in_=oh1[:, :])
            return outh

        def emit_tail(k, outh):
            if k == 0:
                nc.sync.dma_start(
                    out=wo_sb[:, :, :], in_=wo.rearrange("c (n m) -> c n m", m=128)
                )
            bi, qb = divmod(k, nqb)
            g0 = bi * s + qb * QBLK
            # out-projection + bf16 partial writeback (one pipeline stage later
            # than attn, so the normalize chain latency hides under attention)
            ysb = ysb_pool.tile([128, nnch, QBLK], bf16, tag="ysb")
            for n in range(nnch):
                py = pout_pool.tile([128, QBLK], f32, tag="pp")
                nc.tensor.matmul(
                    py[:, :], wo_sb[:, n, :], outh[:, :], start=True, stop=True
                )
                if n % 2 == 0:
                    nc.vector.tensor_copy(ysb[:, n, :], py[:, :])
                else:
                    nc.scalar.copy(out=ysb[:, n, :], in_=py[:, :])
            nc.sync.dma_start(out=yTr[:, :, g0 : g0 + QBLK], in_=ysb[:, :, :])

        # 3-stage pipeline: iteration k emits proj(k), attn(k-1), tail(k-2), so
        # the normalize chain of attn(k-1) hides under tail(k-2) + proj(k+1).
        # At sequence boundaries proj(bi+1, 0) would overwrite kT / vones
        # chunks that attn(bi, nqb-1) still reads, so attn(k-1) goes first.
        qts, fins, ouths = {}, {}, {}
        for k in range(nblk + 2):
            boundary = k % nqb == 0

            def do_attn():
                if 1 <= k <= nblk and (k - 1) in qts:
                    ouths[k - 1] = emit_attn(k - 1, qts.pop(k - 1))

            if boundary:
                do_attn()
            if k < nblk:
                qts[k], fins[k] = emit_proj(k)
            do_attn()
            if k < nblk:
                fins.pop(k)()
            if k >= 2:
                emit_tail(k - 2, ouths.pop(k - 2))


# --------------------------------------------------------------------------
# host entry point
# --------------------------------------------------------------------------

def _shard_inputs(x, attention_mask, w_qkv, w_out, b, s, d):
    xT = np.ascontiguousarray(np.asarray(x, dtype=np.float32).reshape(b * s, d).T)
    w_qkv = np.asarray(w_qkv, dtype=np.float32)
    w_out = np.asarray(w_out, dtype=np.float32)
    cosT, sinTs = _rope_tables_T(s, HD, HPC)
    master = _master_mask(QBLK)
    am = np.asarray(attention_mask)
    # kmT[p, bi, t, 0] = mask value of key position t*128+p in sequence bi
    kmT = np.ascontiguousarray(
        (am != 0).astype(np.float32).reshape(b, s // JCH, JCH).transpose(2, 0, 1)[..., None]
    )
    perm = _perm64()
    cw = HPC * HD  # 128 columns per core
    in_maps = []
    for c in range(NCORES):
        sl = slice(c * cw, (c + 1) * cw)
        wq_c = w_qkv[:, 0 * d :][:, sl].copy()
        wk_c = w_qkv[:, 1 * d :][:, sl].copy()
        wv_c = w_qkv[:, 2 * d :][:, sl].copy()
        # permute q/k columns so rotate-half partners are adjacent partitions
        for h in range(HPC):
            blk = slice(h * HD, (h + 1) * HD)
            wq_c[:, blk] = wq_c[:, blk][:, perm]
            wk_c[:, blk] = wk_c[:, blk][:, perm]
        wqkv_c = np.ascontiguousarray(np.stack([wq_c, wk_c, wv_c], axis=1))  # [d,3,128]
        in_maps.append(
            {
                "xT": xT,
                "wqkv": wqkv_c,
                "wo": np.ascontiguousarray(w_out[sl, :]),
                "cosT": cosT,
                "sinTs": sinTs,
                "master": master,
                "kmT": kmT,
            }
        )
    return in_maps


_PROG_CACHE = {}


def _build_program(b, s, d, mm, has_padding=False):
    key = (b, s, d, mm, has_padding)
    if key in _PROG_CACHE:
        return _PROG_CACHE[key]
    import concourse.mybir as mybir
    from concourse import bacc
    from concourse.tile import TileContext

    f32 = mybir.dt.float32
    bf16 = mybir.dt.bfloat16
    mf = mybir.dt.float32r if mm == "f32r" else f32
    nc = bacc.Bacc("TRN2", target_bir_lowering=False, debug=False)
    bs = b * s
    ins = {
        "xT": nc.dram_tensor("xT", [d, bs], mf, kind="ExternalInput").ap(),
        "wqkv": nc.dram_tensor("wqkv", [d, 3, 128], mf, kind="ExternalInput").ap(),
        "wo": nc.dram_tensor("wo", [128, d], mf, kind="ExternalInput").ap(),
        "cosT": nc.dram_tensor("cosT", [128, s], f32, kind="ExternalInput").ap(),
        "sinTs": nc.dram_tensor("sinTs", [128, s], f32, kind="ExternalInput").ap(),
        "master": nc.dram_tensor("master", [128, 384 + QBLK], f32, kind="ExternalInput").ap(),
        "kmT": nc.dram_tensor("kmT", [128, b, s // JCH, 1], mf, kind="ExternalInput").ap(),
    }
    outs = {"yT": nc.dram_tensor("yT", [d, bs], bf16, kind="ExternalOutput").ap()}
    with TileContext(nc) as tc:
        emit(tc, outs, ins, b=b, s=s, d=d, mm=mm, has_padding=has_padding)
    nc.compile()
    _PROG_CACHE[key] = nc
    return nc


def kernel(x, attention_mask, w_qkv, w_out, *, mm="f32r", trace=False):
    from concourse import bass_utils

    b, s, d = x.shape
    has_padding = bool(np.any(np.asarray(attention_mask) == 0))
    nc = _build_program(b, s, d, mm, has_padding)
    in_maps = _shard_inputs(x, attention_mask, w_qkv, w_out, b, s, d)
    res = bass_utils.run_bass_kernel_spmd(
        nc, in_maps, core_ids=list(range(NCORES)), trace=trace
    )
    acc = res.results[0]["yT"].astype(np.float32)
    for c in range(1, NCORES):
        acc = acc + res.results[c]["yT"].astype(np.float32)
    out = np.ascontiguousarray(acc.T).reshape(b, s, d).astype(np.float32)
    if trace:
        return out, res
    return out
